# revision 7
# baseline (speedup 1.0000x reference)
"""Trainium2 kernel for codebook k-NN retrieval (k=1).

Computes, for each of B=512 queries:
  - argmax over N=65536 of cosine-scores against z_rot_book  -> rot_book label
  - argmax over N of -(||q - b||^2) against z_trans_book     -> trans_book label

Since k == 1, the reference's mode/softmax steps are identities, so the whole
problem reduces to two argmax searches.  Two argmax-preserving simplifications:
  - cosine: the per-query normalization is a positive per-row scale, so raw
    dot products give the same argmax (and are comparable across shards).
  - euclidean: -(q2 + b2 - 2 q.b) has the same argmax as (q.b - 0.5*b2); the
    b2 term is folded in as one extra contraction row (query row = 1.0,
    book row = -0.5*b2).

Sharding: codebooks split along N across 8 cores (8192 entries each), queries
replicated.  Each core computes local top-1 (value, index) per query with the
DVE max / max_index instructions; the host reduces the 8 candidates per query
and gathers the labels.
"""

import os
import sys

import numpy as np

for _p in ("/opt/trn_rl_repo", "/root/.axon_site/_ro/trn_rl_repo"):
    if os.path.isdir(_p) and _p not in sys.path:
        sys.path.insert(0, _p)

B = 512          # queries
N = 65536        # codebook entries
D = 256          # latent dim
NCORES = 8
NS = N // NCORES  # shard size per core: 8192
QCH = B // 128    # query chunks of 128: 4
BLK = 2048        # score column block (one PSUM tile: 4 banks)
NBLK = NS // BLK  # 4

# test.py reads these after calling kernel() with KERNEL_TRACE=1
last_exec_time_ns = None
last_results = None


def _build_program():
    import concourse.mybir as mybir
    from concourse.bacc import Bacc
    from concourse.tile import TileContext

    f32 = mybir.dt.float32
    u32 = mybir.dt.uint32

    # Bacc (not raw Bass): its compile pipeline runs
    # move_matmul_waits_to_ldweights + generate_event_semaphores, which
    # legalize instructions that need >1 semaphore wait (walrus rejects
    # multi-wait Matmults with "Too many sync wait commands").
    nc = Bacc()
    qr = nc.dram_tensor("qr", [D, B], f32, kind="ExternalInput")
    qt = nc.dram_tensor("qt", [D + 1, B], f32, kind="ExternalInput")
    br = nc.dram_tensor("br", [D, NS], f32, kind="ExternalInput")
    bt = nc.dram_tensor("bt", [D + 1, NS], f32, kind="ExternalInput")
    rv = nc.dram_tensor("rv", [QCH, 128], f32, kind="ExternalOutput")
    ri = nc.dram_tensor("ri", [QCH, 128], u32, kind="ExternalOutput")
    tv = nc.dram_tensor("tv", [QCH, 128], f32, kind="ExternalOutput")
    ti = nc.dram_tensor("ti", [QCH, 128], u32, kind="ExternalOutput")

    with TileContext(nc) as tc:
        with (
            tc.tile_pool(name="qpool", bufs=1) as qpool,
            tc.tile_pool(name="bpool", bufs=2) as bpool,
            tc.tile_pool(name="spool", bufs=1) as spool,
            tc.tile_pool(name="opool", bufs=1) as opool,
            tc.tile_pool(name="psum", bufs=2, space="PSUM") as psum_pool,
        ):
            qr0 = qpool.tile([128, B], f32, tag="qr0")
            nc.sync.dma_start(out=qr0[:], in_=qr[0:128, :])
            qr1 = qpool.tile([128, B], f32, tag="qr1")
            nc.sync.dma_start(out=qr1[:], in_=qr[128:256, :])
            qt0 = qpool.tile([128, B], f32, tag="qt0")
            nc.sync.dma_start(out=qt0[:], in_=qt[0:128, :])
            qt1 = qpool.tile([128, B], f32, tag="qt1")
            nc.sync.dma_start(out=qt1[:], in_=qt[128:256, :])
            qt2 = qpool.tile([1, B], f32, tag="qt2")
            nc.sync.dma_start(out=qt2[:], in_=qt[256:257, :])

            for book in range(2):
                qk = (qr0, qr1) if book == 0 else (qt0, qt1)
                bdram = br if book == 0 else bt
                vout, iout = (rv, ri) if book == 0 else (tv, ti)
                scores = [
                    spool.tile([128, NS], f32, tag=f"s{qi}", name=f"s{book}{qi}")
                    for qi in range(QCH)
                ]
                for nb in range(NBLK):
                    csl = slice(nb * BLK, (nb + 1) * BLK)
                    # one DMA (one sem) for both 128-row contraction chunks
                    k01 = bpool.tile([128, 2, BLK], f32, tag="k01")
                    nc.sync.dma_start(
                        out=k01[:],
                        in_=bdram[0:256, csl].rearrange("(t k) n -> k t n", t=2),
                    )
                    if book == 1:
                        b2 = bpool.tile([1, BLK], f32, tag="b2")
                        nc.sync.dma_start(out=b2[:], in_=bdram[256:257, csl])
                    for qi in range(QCH):
                        qsl = slice(qi * 128, (qi + 1) * 128)
                        ps = psum_pool.tile([128, BLK], f32, tag="ps")
                        for ns in range(BLK // 512):
                            sl = slice(ns * 512, (ns + 1) * 512)
                            nc.tensor.matmul(
                                ps[:, sl], qk[0][:, qsl], k01[:, 0, sl],
                                start=True, stop=False, skip_group_check=True,
                            )
                            nc.tensor.matmul(
                                ps[:, sl], qk[1][:, qsl], k01[:, 1, sl],
                                start=False, stop=(book == 0),
                                skip_group_check=True,
                            )
                            if book == 1:
                                nc.tensor.matmul(
                                    ps[:, sl], qt2[0:1, qsl], b2[0:1, sl],
                                    start=False, stop=True,
                                    skip_group_check=True,
                                )
                        nc.scalar.copy(scores[qi][:, csl], ps[:])
                for qi in range(QCH):
                    top8 = opool.tile([128, 8], f32, tag=f"v{book}{qi}")
                    idx8 = opool.tile([128, 8], u32, tag=f"i{book}{qi}")
                    nc.vector.max(out=top8[:], in_=scores[qi][:])
                    nc.vector.max_index(idx8[:], top8[:], scores[qi][:])
                    nc.sync.dma_start(out=vout[qi, :], in_=top8[:, 0:1])
                    nc.sync.dma_start(out=iout[qi, :], in_=idx8[:, 0:1])
    if not nc.is_finalized():
        nc.finalize()
    return nc


def _prep_inputs(z_rot_q, z_trans_q, z_rot_book, z_trans_book):
    qrT = np.ascontiguousarray(z_rot_q.T)
    qtT = np.empty((D + 1, B), np.float32)
    qtT[:D] = z_trans_q.T
    qtT[D] = 1.0
    in_maps = []
    for c in range(NCORES):
        sh = slice(c * NS, (c + 1) * NS)
        brT = np.ascontiguousarray(z_rot_book[sh].T)
        btb = z_trans_book[sh]
        btT = np.empty((D + 1, NS), np.float32)
        btT[:D] = btb.T
        btT[D] = -0.5 * np.einsum("nd,nd->n", btb, btb)
        in_maps.append({"qr": qrT, "qt": qtT, "br": brT, "bt": btT})
    return in_maps


def kernel(z_rot_q, z_trans_q, z_rot_book, z_trans_book, rot_book, trans_book, k):
    global last_exec_time_ns, last_results
    assert int(np.asarray(k)) == 1

    z_rot_q = np.asarray(z_rot_q, np.float32)
    z_trans_q = np.asarray(z_trans_q, np.float32)
    z_rot_book = np.asarray(z_rot_book, np.float32)
    z_trans_book = np.asarray(z_trans_book, np.float32)
    rot_book = np.asarray(rot_book, np.float32)
    trans_book = np.asarray(trans_book, np.float32)

    from concourse.bass_utils import run_bass_kernel_spmd

    nc = _build_program()
    in_maps = _prep_inputs(z_rot_q, z_trans_q, z_rot_book, z_trans_book)
    trace = bool(os.environ.get("KERNEL_TRACE"))
    res = run_bass_kernel_spmd(
        nc, in_maps, core_ids=list(range(NCORES)), trace=trace
    )
    last_exec_time_ns = res.exec_time_ns
    last_results = res

    qidx = np.arange(B)
    out = {}
    for name_v, name_i in (("rv", "ri"), ("tv", "ti")):
        vals = np.stack([r[name_v].reshape(B) for r in res.results])
        idxs = np.stack(
            [r[name_i].reshape(B).astype(np.int64) for r in res.results]
        )
        c = np.argmax(vals, axis=0)
        out[name_i] = c * NS + idxs[c, qidx]

    rotations = rot_book[out["ri"]][:, None, :]      # [B, 1, 1]
    translations = trans_book[out["ti"]]             # [B, 3]
    return rotations, translations


# revision 9
# speedup vs baseline: 1.3854x; 1.3854x over previous
"""Trainium2 kernel for codebook k-NN retrieval (k=1).

Computes, for each of B=512 queries:
  - argmax over N=65536 of cosine-scores against z_rot_book  -> rot_book label
  - argmax over N of -(||q - b||^2) against z_trans_book     -> trans_book label

Since k == 1, the reference's mode/softmax steps are identities, so the whole
problem reduces to two argmax searches.  Two argmax-preserving simplifications:
  - cosine: the per-query normalization is a positive per-row scale, so raw
    dot products give the same argmax (and are comparable across shards).
  - euclidean: -(q2 + b2 - 2 q.b) has the same argmax as (q.b - 0.5*b2); the
    b2 term is folded in as one extra contraction row (query row = 1.0,
    book row = -0.5*b2).

Sharding: codebooks split along N across 8 cores (8192 entries each), queries
replicated.  Each core computes local top-1 (value, index) per query with the
DVE max / max_index instructions; the host reduces the 8 candidates per query
and gathers the labels.
"""

import os
import sys

import numpy as np

for _p in ("/opt/trn_rl_repo", "/root/.axon_site/_ro/trn_rl_repo"):
    if os.path.isdir(_p) and _p not in sys.path:
        sys.path.insert(0, _p)

B = 512          # queries
N = 65536        # codebook entries
D = 256          # latent dim
NCORES = 8
NS = N // NCORES  # shard size per core: 8192
QCH = B // 128    # query chunks of 128: 4
BLK = 2048        # score column block (one PSUM tile: 4 banks)
NBLK = NS // BLK  # 4

# test.py reads these after calling kernel() with KERNEL_TRACE=1
last_exec_time_ns = None
last_results = None


def _build_program():
    import concourse.mybir as mybir
    from concourse.bacc import Bacc
    from concourse.tile import TileContext

    f32 = mybir.dt.float32
    f32r = mybir.dt.float32r
    u32 = mybir.dt.uint32

    # Bacc (not raw Bass): its compile pipeline runs
    # move_matmul_waits_to_ldweights + generate_event_semaphores, which
    # legalize instructions that need >1 semaphore wait (walrus rejects
    # multi-wait Matmults with "Too many sync wait commands").
    nc = Bacc()
    qr = nc.dram_tensor("qr", [D, B], f32r, kind="ExternalInput")
    qt = nc.dram_tensor("qt", [D + 1, B], f32r, kind="ExternalInput")
    br = nc.dram_tensor("br", [D, NS], f32r, kind="ExternalInput")
    bt = nc.dram_tensor("bt", [D + 1, NS], f32r, kind="ExternalInput")
    rv = nc.dram_tensor("rv", [QCH, 128], f32, kind="ExternalOutput")
    ri = nc.dram_tensor("ri", [QCH, 128], u32, kind="ExternalOutput")
    tv = nc.dram_tensor("tv", [QCH, 128], f32, kind="ExternalOutput")
    ti = nc.dram_tensor("ti", [QCH, 128], u32, kind="ExternalOutput")

    with TileContext(nc) as tc:
        with (
            tc.tile_pool(name="qpool", bufs=1) as qpool,
            tc.tile_pool(name="bpool", bufs=2) as bpool,
            tc.tile_pool(name="spool", bufs=1) as spool,
            tc.tile_pool(name="opool", bufs=1) as opool,
            tc.tile_pool(name="psum", bufs=2, space="PSUM") as psum_pool,
        ):
            qr0 = qpool.tile([128, B], f32r, tag="qr0")
            nc.sync.dma_start(out=qr0[:], in_=qr[0:128, :])
            qr1 = qpool.tile([128, B], f32r, tag="qr1")
            nc.sync.dma_start(out=qr1[:], in_=qr[128:256, :])
            qt0 = qpool.tile([128, B], f32r, tag="qt0")
            nc.sync.dma_start(out=qt0[:], in_=qt[0:128, :])
            qt1 = qpool.tile([128, B], f32r, tag="qt1")
            nc.sync.dma_start(out=qt1[:], in_=qt[128:256, :])
            qt2 = qpool.tile([1, B], f32r, tag="qt2")
            nc.sync.dma_start(out=qt2[:], in_=qt[256:257, :])

            for book in range(2):
                qk = (qr0, qr1) if book == 0 else (qt0, qt1)
                bdram = br if book == 0 else bt
                vout, iout = (rv, ri) if book == 0 else (tv, ti)
                scores = [
                    spool.tile([128, NS], f32, tag=f"s{qi}", name=f"s{book}{qi}")
                    for qi in range(QCH)
                ]
                for nb in range(NBLK):
                    csl = slice(nb * BLK, (nb + 1) * BLK)
                    # one DMA (one sem) for both 128-row contraction chunks
                    k01 = bpool.tile([128, 2, BLK], f32r, tag="k01")
                    nc.sync.dma_start(
                        out=k01[:],
                        in_=bdram[0:256, csl].rearrange("(t k) n -> k t n", t=2),
                    )
                    if book == 1:
                        b2 = bpool.tile([1, BLK], f32r, tag="b2")
                        nc.sync.dma_start(out=b2[:], in_=bdram[256:257, csl])
                    for qi in range(QCH):
                        qsl = slice(qi * 128, (qi + 1) * 128)
                        ps = psum_pool.tile([128, BLK], f32, tag="ps")
                        for ns in range(BLK // 512):
                            sl = slice(ns * 512, (ns + 1) * 512)
                            nc.tensor.matmul(
                                ps[:, sl],
                                qk[0][:, qsl],
                                k01[:, 0, sl],
                                start=True, stop=False, skip_group_check=True,
                            )
                            nc.tensor.matmul(
                                ps[:, sl],
                                qk[1][:, qsl],
                                k01[:, 1, sl],
                                start=False, stop=(book == 0),
                                skip_group_check=True,
                            )
                            if book == 1:
                                nc.tensor.matmul(
                                    ps[:, sl],
                                    qt2[0:1, qsl],
                                    b2[0:1, sl],
                                    start=False, stop=True,
                                    skip_group_check=True,
                                )
                        nc.scalar.copy(scores[qi][:, csl], ps[:])
                for qi in range(QCH):
                    top8 = opool.tile([128, 8], f32, tag=f"v{book}{qi}")
                    idx8 = opool.tile([128, 8], u32, tag=f"i{book}{qi}")
                    nc.vector.max(out=top8[:], in_=scores[qi][:])
                    nc.vector.max_index(idx8[:], top8[:], scores[qi][:])
                    nc.sync.dma_start(out=vout[qi, :], in_=top8[:, 0:1])
                    nc.sync.dma_start(out=iout[qi, :], in_=idx8[:, 0:1])
    if not nc.is_finalized():
        nc.finalize()
    return nc


def _prep_inputs(z_rot_q, z_trans_q, z_rot_book, z_trans_book):
    qrT = np.ascontiguousarray(z_rot_q.T)
    qtT = np.empty((D + 1, B), np.float32)
    qtT[:D] = z_trans_q.T
    qtT[D] = 1.0
    in_maps = []
    for c in range(NCORES):
        sh = slice(c * NS, (c + 1) * NS)
        brT = np.ascontiguousarray(z_rot_book[sh].T)
        btb = z_trans_book[sh]
        btT = np.empty((D + 1, NS), np.float32)
        btT[:D] = btb.T
        btT[D] = -0.5 * np.einsum("nd,nd->n", btb, btb)
        in_maps.append({"qr": qrT, "qt": qtT, "br": brT, "bt": btT})
    return in_maps


def kernel(z_rot_q, z_trans_q, z_rot_book, z_trans_book, rot_book, trans_book, k):
    global last_exec_time_ns, last_results
    assert int(np.asarray(k)) == 1

    z_rot_q = np.asarray(z_rot_q, np.float32)
    z_trans_q = np.asarray(z_trans_q, np.float32)
    z_rot_book = np.asarray(z_rot_book, np.float32)
    z_trans_book = np.asarray(z_trans_book, np.float32)
    rot_book = np.asarray(rot_book, np.float32)
    trans_book = np.asarray(trans_book, np.float32)

    from concourse.bass_utils import run_bass_kernel_spmd

    nc = _build_program()
    in_maps = _prep_inputs(z_rot_q, z_trans_q, z_rot_book, z_trans_book)
    trace = bool(os.environ.get("KERNEL_TRACE"))
    res = run_bass_kernel_spmd(
        nc, in_maps, core_ids=list(range(NCORES)), trace=trace
    )
    last_exec_time_ns = res.exec_time_ns
    last_results = res

    qidx = np.arange(B)
    out = {}
    for name_v, name_i in (("rv", "ri"), ("tv", "ti")):
        vals = np.stack([r[name_v].reshape(B) for r in res.results])
        idxs = np.stack(
            [r[name_i].reshape(B).astype(np.int64) for r in res.results]
        )
        c = np.argmax(vals, axis=0)
        out[name_i] = c * NS + idxs[c, qidx]

    rotations = rot_book[out["ri"]][:, None, :]      # [B, 1, 1]
    translations = trans_book[out["ti"]]             # [B, 3]
    return rotations, translations
